# revision 15
# baseline (speedup 1.0000x reference)
"""Causal self-attention (B=2, T=2048, C=1024, H=16) for 8 trn2 NeuronCores.

Reference semantics (note the no-transpose reshape):
    q = (x @ Wq.T).reshape(B, H, T, Dh)   # head h <- rows [128h, 128h+128) of x@Wq.T
so head (b, h) depends only on the 128-row block x[b, 128h:128h+128, :].
The 32 (b, h) pairs are fully independent -> 4 per core (pure data parallel).

Per-core device pipeline (bf16 matmul operands, fp32 PSUM accumulation;
bf16 lowers to separate LDWEIGHTS instructions that the PE's 64-deep
reorder window hides under the previous matmul's streaming, unlike
fp32/f32r fused self-loading matmuls which serialize the weight load):
  - host pre-transposes x blocks and weights so the contraction dim (c) is
    on partitions for both matmul operands, and pre-casts to bf16.
  - per head (interleaved so ScalarE exp work starts ~12us in instead of
    after all projections - ScalarE is the steady-state critical engine):
    Q/K projections computed directly transposed: lhsT = W^T column chunk
    [128c, 128], rhs = x^T block (N=128); 4 chunks packed per PSUM bank,
    then one wide strided copy per (slab, bank) into QT/KT [64, 2048] in
    t2 = 16*i + r order.
    V projection natural [128, 1024], DMA-reshuffled into Vt tiles
    [s2=128, 65] (flat-stream DMA; col 64 = ones -> softmax denominator).
    attention transposed over t2 halves: scoresT[s2, t2] = KT_tile.T @ QT
    (tiles above the causal diagonal skipped), one exp per (half, s) via
    ScalarE (scale=1/8 folded in; no max subtraction needed: |scores| <~ 8),
    triangular mask multiplied on the diagonal 128-tile only, then
    outT[65, t2] += Vt[s].T @ p accumulated in PSUM.
  - host divides by the denominator row and transposes/reshapes back
    (0.007% of the FLOPs; pure unsharding layout work).
"""
import numpy as np

import concourse.bacc as bacc
import concourse.tile as tile
from concourse import mybir
from concourse import bass_utils

B, T, C = 2, 2048, 1024
H, Dh = 16, 64
N_CORES = 8
HPC = 4  # (b, h) pairs per core
ATT_DT = mybir.dt.bfloat16
EXP_SCALE = 1.0 / np.sqrt(Dh)

_CACHED_NC = None


def _chunks(c0, w, h0):
    """Split [c0, c0+w) into pieces <=512 that don't straddle 512-aligned
    PSUM bank boundaries relative to the tile start h0."""
    out = []
    off = 0
    while off < w:
        pos = c0 - h0 + off
        n = min(512 - pos % 512, w - off)
        out.append((pos, c0 + off, n))
        off += n
    return out


def build_nc():
    nc = bacc.Bacc("TRN2", target_bir_lowering=False, debug=False)

    xt_d = nc.dram_tensor("xt", (8, 128, HPC, 128), ATT_DT, kind="ExternalInput")
    w_d = {
        w: nc.dram_tensor(w, (8, 128, 1024), ATT_DT, kind="ExternalInput")
        for w in ("wq", "wk", "wv")
    }
    mask_d = nc.dram_tensor("mask", (128, 128), ATT_DT, kind="ExternalInput")
    out_d = nc.dram_tensor("out", (HPC, Dh + 1, T), mybir.dt.float32, kind="ExternalOutput")

    with tile.TileContext(nc) as tc, \
         tc.tile_pool(name="wts", bufs=1) as wts, \
         tc.tile_pool(name="work", bufs=2) as work, \
         tc.tile_pool(name="ps_proj", bufs=2, space="PSUM") as ps_proj, \
         tc.tile_pool(name="ps_sc", bufs=2, space="PSUM") as ps_sc, \
         tc.tile_pool(name="ps_av", bufs=1, space="PSUM") as ps_av:

        # ---- resident inputs (all bf16: 48KB/partition for 3 weights) ----
        mask_s = wts.tile([128, 128], ATT_DT, tag="mask")
        nc.sync.dma_start(mask_s[:], mask_d.ap())
        xt_s = []
        for ct in range(8):
            t = wts.tile([128, HPC, 128], ATT_DT, tag=f"xt{ct}", name=f"xt{ct}")
            nc.sync.dma_start(t[:], xt_d.ap()[ct])
            xt_s.append(t)
        wv_s = []
        for ct in range(8):
            t = wts.tile([128, 1024], ATT_DT, tag=f"wv{ct}", name=f"wv{ct}")
            nc.sync.dma_start(t[:], w_d["wv"].ap()[ct])
            wv_s.append(t)

        # ---- Q/K projections, all 4 heads batched, stored slab-major ----
        # psum fill g: partitions = W^T cols [128g, 128g+128) = slabs r=2g,2g+1
        # (slab r holds q[., r*64:(r+1)*64] of the natural projection);
        # cols = 4 heads x 128 block rows i.  Storage column = 512r+128hl+i;
        # logical t2 = 16i + r is produced by strided APs in the scores MMs.
        # Q/K stored with a PERMUTED within-128-block t2 order: storage column
        # = 2048*hl + 128*blk + 8*r + il  for t2 = 16*(8*blk+il) + r.  Causal
        # tile structure only depends on 128-blocks, so any fixed within-block
        # permutation works as long as Q cols, K cols (= p partitions = Vt
        # rows) and the diagonal mask are consistent; the host unscrambles
        # output columns.  This makes the PSUM->SBUF projection copies
        # 8-element-contiguous (fast) instead of stride-16 scatters (~2us
        # each, which stalled the PE between fills and kept HAM cold).
        qt_all = wts.tile([64, HPC * T], ATT_DT, tag="qt", name="qt_all")
        kt_all = wts.tile([64, HPC * T], ATT_DT, tag="kt", name="kt_all")
        # Q: permuted storage, fast 8-contiguous DVE copies.  K: natural t2
        # order (stationary APs must be one free dim), scatter copies on the
        # otherwise-idle ScalarE so the DVE doesn't stall the projections.
        qt_view = qt_all.rearrange("d (hl k r il) -> d r hl k il",
                                   hl=HPC, k=16, r=16, il=8)
        kt_view = kt_all.rearrange("d (hl i r) -> d r hl i",
                                   hl=HPC, i=128, r=16)
        for w in ("wq", "wk"):
            for g in range(8):
                wslab = work.tile([128, 8, 128], ATT_DT, tag="wslab",
                                  name=f"wslab_{w}{g}", bufs=4)
                for ct in range(8):
                    nc.sync.dma_start(
                        wslab[:, ct, :],
                        w_d[w].ap()[ct, :, 128 * g:128 * (g + 1)],
                    )
                ps = ps_proj.tile([128, 512], mybir.dt.float32, tag="proj")
                for ct in range(8):
                    nc.tensor.matmul(
                        ps[:],
                        wslab[:, ct, :],
                        xt_s[ct][:],
                        start=(ct == 0),
                        stop=(ct == 7),
                    )
                for a in range(2):
                    r = 2 * g + a
                    if w == "wq":
                        nc.vector.tensor_copy(
                            qt_view[:, r],
                            ps[64 * a:64 * a + 64, :].rearrange(
                                "d (hl k il) -> d hl k il", hl=HPC, k=16),
                        )
                    else:
                        nc.scalar.copy(
                            kt_view[:, r],
                            ps[64 * a:64 * a + 64, :].rearrange(
                                "d (hl i) -> d hl i", hl=HPC),
                        )

        # ---- per head: V projection + reshuffle, then attention ----
        for hl in range(HPC):
            # V projection, natural layout
            vblk = work.tile([128, 1024], ATT_DT, tag="vblk", name="vblk")
            for jc in range(2):
                ps = ps_proj.tile([128, 512], mybir.dt.float32, tag="proj")
                for ct in range(8):
                    nc.tensor.matmul(
                        ps[:],
                        xt_s[ct][:, hl, :],
                        wv_s[ct][:, 512 * jc:512 * (jc + 1)],
                        start=(ct == 0),
                        stop=(ct == 7),
                    )
                nc.vector.tensor_copy(vblk[:, 512 * jc:512 * (jc + 1)], ps[:])

            # Vt[s2, 16, 65]: flat-stream DMA does [8, 1024] -> [128, 64]
            vtil = work.tile([128, 16, Dh + 1], ATT_DT, tag="vtil", name="vtil")
            nc.gpsimd.memset(vtil[:, :, Dh:Dh + 1], 1.0)
            for k in range(16):
                nc.sync.dma_start(
                    out=vtil[:, k, 0:Dh], in_=vblk[8 * k:8 * (k + 1), :]
                )

            # ---- attention over t2 halves [h0, h0+1024) ----
            for q2 in range(2):
                h0 = 1024 * q2
                av = ps_av.tile([Dh + 1, 1024], mybir.dt.float32, tag="av")
                smax = 8 * q2 + 7
                for s in range(smax + 1):
                    c0 = max(h0, 128 * s)
                    w = h0 + 1024 - c0
                    sc = ps_sc.tile([128, 1024], mybir.dt.float32, tag="sc")
                    for (pos, t2, n) in _chunks(c0, w, h0):
                        nc.tensor.matmul(
                            sc[:, pos:pos + n],
                            kt_all[:, T * hl + 128 * s:T * hl + 128 * s + 128],
                            qt_all[:, T * hl + t2:T * hl + t2 + n],
                            start=True,
                            stop=True,
                        )
                    p = work.tile([128, 1024], ATT_DT, tag="p", name="p", bufs=3)
                    nc.scalar.activation(
                        p[:, c0 - h0:], sc[:, c0 - h0:],
                        mybir.ActivationFunctionType.Exp, scale=float(EXP_SCALE),
                    )
                    if 128 * s >= h0:  # diagonal 128-tile needs triangular mask
                        nc.vector.tensor_mul(
                            p[:, c0 - h0:c0 - h0 + 128],
                            p[:, c0 - h0:c0 - h0 + 128],
                            mask_s[:],
                        )
                    for (pos, t2, n) in _chunks(c0, w, h0):
                        nc.tensor.matmul(
                            av[:, pos:pos + n],
                            vtil[:, s, :],
                            p[:, pos:pos + n],
                            start=(s == 0),
                            stop=(s == smax),
                        )
                ot = work.tile([Dh + 1, 1024], mybir.dt.float32, tag="ot", name="ot")
                nc.vector.tensor_copy(ot[:], av[:])
                nc.sync.dma_start(out_d.ap()[hl, :, h0:h0 + 1024], ot[:])

    nc.compile()
    return nc


def prepare_in_maps(x, Wq, Wk, Wv):
    np_dt = mybir.dt.np(ATT_DT)
    x = np.asarray(x, dtype=np.float32)
    wts = {w: np.ascontiguousarray(np.asarray(a, np.float32).T)
           .reshape(8, 128, 1024).astype(np_dt)
           for w, a in (("wq", Wq), ("wk", Wk), ("wv", Wv))}
    # within-128-block permuted order: index u = 8r + il <-> t2off = 16*il + r
    u = np.arange(128)
    t2off = 16 * (u % 8) + u // 8  # permuted Q/out column u -> within-block t2
    mask = (u[:, None] <= t2off[None, :]).astype(np.float32).astype(np_dt)

    in_maps = []
    for core in range(N_CORES):
        xt = np.empty((8, 128, HPC, 128), dtype=np_dt)
        for hl in range(HPC):
            bh = core * HPC + hl
            b, h = divmod(bh, H)
            blk_t = np.ascontiguousarray(x[b, 128 * h:128 * (h + 1), :].T)  # (1024, 128)
            xt[:, :, hl, :] = blk_t.reshape(8, 128, 128).astype(np_dt)
        m = {"xt": xt, "mask": mask}
        m.update(wts)
        in_maps.append(m)
    return in_maps


_T2 = np.arange(T)
_DEVCOL_OF_T2 = 128 * (_T2 // 128) + 8 * (_T2 % 16) + (_T2 // 16) % 8


def gather_output(results):
    y = np.empty((B, T, C), dtype=np.float32)
    for core in range(N_CORES):
        o = results[core]["out"]  # (HPC, 65, 2048)
        for hl in range(HPC):
            bh = core * HPC + hl
            b, h = divmod(bh, H)
            ob = o[hl][:, _DEVCOL_OF_T2]  # undo within-block permutation
            blk = (ob[:Dh] / ob[Dh:Dh + 1]).T  # (2048, 64)
            y[b, 128 * h:128 * (h + 1), :] = blk.reshape(128, 1024)
    return y


def get_nc():
    global _CACHED_NC
    if _CACHED_NC is None:
        _CACHED_NC = build_nc()
    return _CACHED_NC


def kernel(x, Wq, Wk, Wv):
    in_maps = prepare_in_maps(x, Wq, Wk, Wv)
    res = bass_utils.run_bass_kernel_spmd(
        get_nc(), in_maps, core_ids=list(range(N_CORES))
    )
    return gather_output(res.results)


# revision 16
# speedup vs baseline: 1.0290x; 1.0290x over previous
"""Causal self-attention (B=2, T=2048, C=1024, H=16) for 8 trn2 NeuronCores.

Reference semantics (note the no-transpose reshape):
    q = (x @ Wq.T).reshape(B, H, T, Dh)   # head h <- rows [128h, 128h+128) of x@Wq.T
so head (b, h) depends only on the 128-row block x[b, 128h:128h+128, :].
The 32 (b, h) pairs are fully independent -> 4 per core (pure data parallel).

Per-core device pipeline (bf16 matmul operands, fp32 PSUM accumulation;
bf16 lowers to separate LDWEIGHTS instructions that the PE's 64-deep
reorder window hides under the previous matmul's streaming, unlike
fp32/f32r fused self-loading matmuls which serialize the weight load):
  - host pre-transposes x blocks and weights so the contraction dim (c) is
    on partitions for both matmul operands, and pre-casts to bf16.
  - per head (interleaved so ScalarE exp work starts ~12us in instead of
    after all projections - ScalarE is the steady-state critical engine):
    Q/K projections computed directly transposed: lhsT = W^T column chunk
    [128c, 128], rhs = x^T block (N=128); 4 chunks packed per PSUM bank,
    then one wide strided copy per (slab, bank) into QT/KT [64, 2048] in
    t2 = 16*i + r order.
    V projection natural [128, 1024], DMA-reshuffled into Vt tiles
    [s2=128, 65] (flat-stream DMA; col 64 = ones -> softmax denominator).
    attention transposed over t2 halves: scoresT[s2, t2] = KT_tile.T @ QT
    (tiles above the causal diagonal skipped), one exp per (half, s) via
    ScalarE (scale=1/8 folded in; no max subtraction needed: |scores| <~ 8),
    triangular mask multiplied on the diagonal 128-tile only, then
    outT[65, t2] += Vt[s].T @ p accumulated in PSUM.
  - host divides by the denominator row and transposes/reshapes back
    (0.007% of the FLOPs; pure unsharding layout work).
"""
import numpy as np

import concourse.bacc as bacc
import concourse.tile as tile
from concourse import mybir
from concourse import bass_utils

B, T, C = 2, 2048, 1024
H, Dh = 16, 64
N_CORES = 8
HPC = 4  # (b, h) pairs per core
ATT_DT = mybir.dt.bfloat16
EXP_SCALE = 1.0 / np.sqrt(Dh)

_CACHED_NC = None


def _chunks(c0, w, h0):
    """Split [c0, c0+w) into pieces <=512 that don't straddle 512-aligned
    PSUM bank boundaries relative to the tile start h0."""
    out = []
    off = 0
    while off < w:
        pos = c0 - h0 + off
        n = min(512 - pos % 512, w - off)
        out.append((pos, c0 + off, n))
        off += n
    return out


def build_nc():
    nc = bacc.Bacc("TRN2", target_bir_lowering=False, debug=False)

    xt_d = nc.dram_tensor("xt", (8, 128, HPC, 128), ATT_DT, kind="ExternalInput")
    w_d = {
        w: nc.dram_tensor(w, (8, 128, 1024), ATT_DT, kind="ExternalInput")
        for w in ("wq", "wk", "wv")
    }
    mask_d = nc.dram_tensor("mask", (128, 128), ATT_DT, kind="ExternalInput")
    out_d = nc.dram_tensor("out", (HPC, Dh + 1, T), mybir.dt.float32, kind="ExternalOutput")

    with tile.TileContext(nc) as tc, \
         tc.tile_pool(name="wts", bufs=1) as wts, \
         tc.tile_pool(name="work", bufs=2) as work, \
         tc.tile_pool(name="ps_proj", bufs=2, space="PSUM") as ps_proj, \
         tc.tile_pool(name="ps_sc", bufs=2, space="PSUM") as ps_sc, \
         tc.tile_pool(name="ps_av", bufs=1, space="PSUM") as ps_av:

        # ---- resident inputs; only xt blocks the first matmuls, so wv and
        # the mask are DMA'd after the QK-proj stream is emitted ----
        xt_s = []
        for ct in range(8):
            t = wts.tile([128, HPC, 128], ATT_DT, tag=f"xt{ct}", name=f"xt{ct}")
            nc.sync.dma_start(t[:], xt_d.ap()[ct])
            xt_s.append(t)

        # ---- Q/K projections, all 4 heads batched, stored slab-major ----
        # psum fill g: partitions = W^T cols [128g, 128g+128) = slabs r=2g,2g+1
        # (slab r holds q[., r*64:(r+1)*64] of the natural projection);
        # cols = 4 heads x 128 block rows i.  Storage column = 512r+128hl+i;
        # logical t2 = 16i + r is produced by strided APs in the scores MMs.
        # Q/K stored with a PERMUTED within-128-block t2 order: storage column
        # = 2048*hl + 128*blk + 8*r + il  for t2 = 16*(8*blk+il) + r.  Causal
        # tile structure only depends on 128-blocks, so any fixed within-block
        # permutation works as long as Q cols, K cols (= p partitions = Vt
        # rows) and the diagonal mask are consistent; the host unscrambles
        # output columns.  This makes the PSUM->SBUF projection copies
        # 8-element-contiguous (fast) instead of stride-16 scatters (~2us
        # each, which stalled the PE between fills and kept HAM cold).
        qt_all = wts.tile([64, HPC * T], ATT_DT, tag="qt", name="qt_all")
        kt_all = wts.tile([64, HPC * T], ATT_DT, tag="kt", name="kt_all")
        # Q: permuted storage, fast 8-contiguous DVE copies.  K: natural t2
        # order (stationary APs must be one free dim), scatter copies on the
        # otherwise-idle ScalarE so the DVE doesn't stall the projections.
        qt_view = qt_all.rearrange("d (hl k r il) -> d r hl k il",
                                   hl=HPC, k=16, r=16, il=8)
        kt_view = kt_all.rearrange("d (hl i r) -> d r hl i",
                                   hl=HPC, i=128, r=16)
        for w in ("wq", "wk"):
            for g in range(8):
                wslab = work.tile([128, 8, 128], ATT_DT, tag="wslab",
                                  name=f"wslab_{w}{g}", bufs=4)
                for ct in range(8):
                    nc.sync.dma_start(
                        wslab[:, ct, :],
                        w_d[w].ap()[ct, :, 128 * g:128 * (g + 1)],
                    )
                ps = ps_proj.tile([128, 512], mybir.dt.float32, tag="proj")
                for ct in range(8):
                    nc.tensor.matmul(
                        ps[:],
                        wslab[:, ct, :],
                        xt_s[ct][:],
                        start=(ct == 0),
                        stop=(ct == 7),
                    )
                for a in range(2):
                    r = 2 * g + a
                    if w == "wq":
                        nc.vector.tensor_copy(
                            qt_view[:, r],
                            ps[64 * a:64 * a + 64, :].rearrange(
                                "d (hl k il) -> d hl k il", hl=HPC, k=16),
                        )
                    else:
                        eng = nc.vector.tensor_copy if a == 0 else nc.scalar.copy
                        eng(
                            kt_view[:, r],
                            ps[64 * a:64 * a + 64, :].rearrange(
                                "d (hl i) -> d hl i", hl=HPC),
                        )

        wv_s = []
        for ct in range(8):
            t = wts.tile([128, 1024], ATT_DT, tag=f"wv{ct}", name=f"wv{ct}")
            nc.sync.dma_start(t[:], w_d["wv"].ap()[ct])
            wv_s.append(t)
        mask_s = wts.tile([128, 128], ATT_DT, tag="mask")
        nc.sync.dma_start(mask_s[:], mask_d.ap())

        # ---- per head: V projection + reshuffle, then attention ----
        for hl in range(HPC):
            # V projection, natural layout
            vblk = work.tile([128, 1024], ATT_DT, tag="vblk", name="vblk")
            for jc in range(2):
                ps = ps_proj.tile([128, 512], mybir.dt.float32, tag="proj")
                for ct in range(8):
                    nc.tensor.matmul(
                        ps[:],
                        xt_s[ct][:, hl, :],
                        wv_s[ct][:, 512 * jc:512 * (jc + 1)],
                        start=(ct == 0),
                        stop=(ct == 7),
                    )
                nc.vector.tensor_copy(vblk[:, 512 * jc:512 * (jc + 1)], ps[:])

            # Vt[s2, 16, 65]: flat-stream DMA does [8, 1024] -> [128, 64]
            vtil = work.tile([128, 16, Dh + 1], ATT_DT, tag="vtil", name="vtil")
            nc.gpsimd.memset(vtil[:, :, Dh:Dh + 1], 1.0)
            for k in range(16):
                nc.sync.dma_start(
                    out=vtil[:, k, 0:Dh], in_=vblk[8 * k:8 * (k + 1), :]
                )

            # ---- attention over t2 halves [h0, h0+1024) ----
            for q2 in range(2):
                h0 = 1024 * q2
                av = ps_av.tile([Dh + 1, 1024], mybir.dt.float32, tag="av")
                smax = 8 * q2 + 7
                for s in range(smax + 1):
                    c0 = max(h0, 128 * s)
                    w = h0 + 1024 - c0
                    sc = ps_sc.tile([128, 1024], mybir.dt.float32, tag="sc")
                    for (pos, t2, n) in _chunks(c0, w, h0):
                        nc.tensor.matmul(
                            sc[:, pos:pos + n],
                            kt_all[:, T * hl + 128 * s:T * hl + 128 * s + 128],
                            qt_all[:, T * hl + t2:T * hl + t2 + n],
                            start=True,
                            stop=True,
                        )
                    p = work.tile([128, 1024], ATT_DT, tag="p", name="p", bufs=3)
                    nc.scalar.activation(
                        p[:, c0 - h0:], sc[:, c0 - h0:],
                        mybir.ActivationFunctionType.Exp, scale=float(EXP_SCALE),
                    )
                    if 128 * s >= h0:  # diagonal 128-tile needs triangular mask
                        nc.vector.tensor_mul(
                            p[:, c0 - h0:c0 - h0 + 128],
                            p[:, c0 - h0:c0 - h0 + 128],
                            mask_s[:],
                        )
                    for (pos, t2, n) in _chunks(c0, w, h0):
                        nc.tensor.matmul(
                            av[:, pos:pos + n],
                            vtil[:, s, :],
                            p[:, pos:pos + n],
                            start=(s == 0),
                            stop=(s == smax),
                        )
                ot = work.tile([Dh + 1, 1024], mybir.dt.float32, tag="ot", name="ot")
                nc.vector.tensor_copy(ot[:], av[:])
                nc.sync.dma_start(out_d.ap()[hl, :, h0:h0 + 1024], ot[:])

    nc.compile()
    return nc


def prepare_in_maps(x, Wq, Wk, Wv):
    np_dt = mybir.dt.np(ATT_DT)
    x = np.asarray(x, dtype=np.float32)
    wts = {w: np.ascontiguousarray(np.asarray(a, np.float32).T)
           .reshape(8, 128, 1024).astype(np_dt)
           for w, a in (("wq", Wq), ("wk", Wk), ("wv", Wv))}
    # within-128-block permuted order: index u = 8r + il <-> t2off = 16*il + r
    u = np.arange(128)
    t2off = 16 * (u % 8) + u // 8  # permuted Q/out column u -> within-block t2
    mask = (u[:, None] <= t2off[None, :]).astype(np.float32).astype(np_dt)

    in_maps = []
    for core in range(N_CORES):
        xt = np.empty((8, 128, HPC, 128), dtype=np_dt)
        for hl in range(HPC):
            bh = core * HPC + hl
            b, h = divmod(bh, H)
            blk_t = np.ascontiguousarray(x[b, 128 * h:128 * (h + 1), :].T)  # (1024, 128)
            xt[:, :, hl, :] = blk_t.reshape(8, 128, 128).astype(np_dt)
        m = {"xt": xt, "mask": mask}
        m.update(wts)
        in_maps.append(m)
    return in_maps


_T2 = np.arange(T)
_DEVCOL_OF_T2 = 128 * (_T2 // 128) + 8 * (_T2 % 16) + (_T2 // 16) % 8


def gather_output(results):
    y = np.empty((B, T, C), dtype=np.float32)
    for core in range(N_CORES):
        o = results[core]["out"]  # (HPC, 65, 2048)
        for hl in range(HPC):
            bh = core * HPC + hl
            b, h = divmod(bh, H)
            ob = o[hl][:, _DEVCOL_OF_T2]  # undo within-block permutation
            blk = (ob[:Dh] / ob[Dh:Dh + 1]).T  # (2048, 64)
            y[b, 128 * h:128 * (h + 1), :] = blk.reshape(128, 1024)
    return y


def get_nc():
    global _CACHED_NC
    if _CACHED_NC is None:
        _CACHED_NC = build_nc()
    return _CACHED_NC


def kernel(x, Wq, Wk, Wv):
    in_maps = prepare_in_maps(x, Wq, Wk, Wv)
    res = bass_utils.run_bass_kernel_spmd(
        get_nc(), in_maps, core_ids=list(range(N_CORES))
    )
    return gather_output(res.results)
